# revision 4
# baseline (speedup 1.0000x reference)
"""ClusterPooling kernel for 8x Trainium2 NeuronCores (Bass/Tile).

Decomposition (validated against the jax reference):
  e(u,v) = tanh(p_u + q_v + b),  p = x @ w[:F], q = x @ w[F:]
  A   = symmetric 0/1 adjacency (diag 0)        [dense, needed on device]
  S   = A * tanh(p+q+b) outer-sum mask          [dense, device]
  A_c = directed contracted adjacency           [sparse, host: sign tests only]
  labels = reference's directed min-label fixed point  [sparse O(E), host]
  cluster/C = consecutive relabel one-hot       [device builds C from ids]
  X_new = (S@C).T @ x ; A_new = C.T @ A @ C     [dense matmuls, device]

Sharding: core r owns columns R_r = [r*384, (r+1)*384) of the dense
matrices (column shard of A == row shard of A^T; A symmetric).  Each core
computes [256,128] / [256,256] partials; host sums the 8 partials and
embeds into the padded full-size outputs.
"""
import numpy as np

N, F, E = 3072, 128, 24576
NCORES = 8
SH = N // NCORES          # 384 shard columns per core
NCH = N // 128            # 24 partition chunks of the full dim
MCH = SH // 128           # 3 partition chunks of the shard dim
KPAD = 256                # padded cluster count (2 x 128)

_PROG = None              # cached compiled program


# ---------------------------------------------------------------- host math
def _host_prep(x, edge_index, batch, w, b):
    x = np.asarray(x, dtype=np.float32)
    w = np.asarray(w, dtype=np.float32)
    ei = np.asarray(edge_index)
    b = np.float32(np.asarray(b))

    src = ei[0].astype(np.int64)
    dst = ei[1].astype(np.int64)
    s2 = np.concatenate([src, dst])
    d2 = np.concatenate([dst, src])

    # Sign-critical per-edge scores: replicate the reference's exact ops on
    # jax CPU so threshold decisions bit-match the oracle.
    try:
        import jax
        import jax.numpy as jnp
        cpu = jax.devices("cpu")[0]
        with jax.default_device(cpu):
            xj = jnp.asarray(x)
            wj = jnp.asarray(w)
            t = xj[s2] @ wj[:F] + xj[d2] @ wj[F:] + jnp.asarray(b)
            contract = np.asarray(t > 0.0)
    except Exception:
        p_ = x @ w[:F]
        q_ = x @ w[F:]
        contract = (p_[s2] + q_[d2] + float(b)) > 0.0

    p = x @ w[:F]
    q = x @ w[F:]

    # symmetric adjacency mask, diag zero
    A = np.zeros((N, N), dtype=np.uint8)
    A[s2, d2] = 1
    np.fill_diagonal(A, 0)

    keep = contract & (s2 != d2)
    cs, cd = s2[keep], d2[keep]

    # reference's directed min-label propagation + pointer jumping, exactly
    labels = np.arange(N, dtype=np.int64)
    while True:
        neigh = np.full(N, N, dtype=np.int64)
        np.minimum.at(neigh, cs, labels[cd])
        new = np.minimum(labels, neigh)
        new = np.minimum(new, new[new])
        if np.array_equal(new, labels):
            break
        labels = new

    roots = (labels == np.arange(N)).astype(np.int64)
    cluster = (np.cumsum(roots) - 1)[labels]
    K = int(cluster.max()) + 1

    deg = np.zeros(N, dtype=np.int64)
    np.add.at(deg, cs, 1)
    np.add.at(deg, cd, 1)
    single = (deg == 0).astype(np.float32)

    return p, q, A, cluster, K, single


def _numpy_fallback(x, batch, p, q, b, A, cluster, single):
    """Dense-on-host path, used only if K > KPAD (never for the fixed input)."""
    K = int(cluster.max()) + 1
    C = np.zeros((N, K), dtype=np.float32)
    C[np.arange(N), cluster] = 1.0
    Tm = np.tanh((p + float(b))[None, :] + q[:, None]).astype(np.float32)
    M = A.astype(np.float32).T * Tm            # M[j,i] = S[i,j]
    W = M.T @ C + single[:, None] * C
    X = W.T @ x.astype(np.float32)
    An = C.T @ A.astype(np.float32) @ C
    np.fill_diagonal(An, 0.0)
    X_new = np.zeros((N, F), dtype=np.float32)
    X_new[:K] = X
    A_new = np.zeros((N, N), dtype=np.float32)
    A_new[:K, :K] = An
    new_batch = np.zeros(N, dtype=np.asarray(batch).dtype)
    np.maximum.at(new_batch, cluster, np.asarray(batch))
    return X_new, A_new, new_batch, cluster.astype(np.int32)


# ------------------------------------------------------------- bass program
def _build_program():
    import concourse.bacc as bacc
    import concourse.tile as tile
    import concourse.mybir as mybir

    f32 = mybir.dt.float32
    bf16 = mybir.dt.bfloat16
    u8 = mybir.dt.uint8

    nc = bacc.Bacc("TRN2", target_bir_lowering=False, debug=False,
                   num_devices=NCORES)

    acol8 = nc.dram_tensor("acol8", [128, NCH, SH], u8, kind="ExternalInput").ap()
    xsh = nc.dram_tensor("xsh", [128, MCH, F], f32, kind="ExternalInput").ap()
    ppmat = nc.dram_tensor("ppmat", [128, SH], f32, kind="ExternalInput").ap()
    qmat = nc.dram_tensor("qmat", [128, NCH], f32, kind="ExternalInput").ap()
    clumat = nc.dram_tensor("clumat", [128, NCH], f32, kind="ExternalInput").ap()
    clush = nc.dram_tensor("clush", [128, MCH], f32, kind="ExternalInput").ap()
    snglsh = nc.dram_tensor("snglsh", [128, MCH], f32, kind="ExternalInput").ap()

    xpart = nc.dram_tensor("xpart", [128, 2, F], f32, kind="ExternalOutput").ap()
    anpart = nc.dram_tensor("anpart", [128, 2, KPAD], f32, kind="ExternalOutput").ap()

    with tile.TileContext(nc) as tc:
        with (
            tc.tile_pool(name="persist", bufs=1) as pp,
            tc.tile_pool(name="pipe", bufs=3) as pipe,
            tc.tile_pool(name="psum", bufs=2, space="PSUM") as ps,
        ):
            # ---- small inputs
            pp_sb = pp.tile([128, SH], f32, tag="ppmat", name="ppmat")
            q_sb = pp.tile([128, NCH], f32, tag="qmat", name="qmat")
            clu_sb = pp.tile([128, NCH], f32, tag="clumat", name="clumat")
            clush_sb = pp.tile([128, MCH], f32, tag="clush", name="clush")
            sngl_sb = pp.tile([128, MCH], f32, tag="snglsh", name="snglsh")
            x_sb = pp.tile([128, MCH, F], f32, tag="xsh", name="xsh")
            nc.sync.dma_start(pp_sb[:], ppmat[:])
            nc.sync.dma_start(q_sb[:], qmat[:])
            nc.sync.dma_start(clu_sb[:], clumat[:])
            nc.sync.dma_start(clush_sb[:], clush[:])
            nc.sync.dma_start(sngl_sb[:], snglsh[:])
            nc.sync.dma_start(x_sb[:], xsh[:])

            # ---- one-hot C from cluster ids: C[p, k] = (iota_k == clu[p])
            iota_sb = pp.tile([128, KPAD], f32, tag="iota", name="iota")
            nc.gpsimd.iota(iota_sb[:], pattern=[[1, KPAD]], base=0,
                           channel_multiplier=0,
                           allow_small_or_imprecise_dtypes=True)

            Cf = [pp.tile([128, KPAD], f32, tag=f"Cf{c}", name=f"Cf{c}") for c in range(NCH)]
            Cb = [pp.tile([128, KPAD], bf16, tag=f"Cb{c}", name=f"Cb{c}") for c in range(NCH)]
            for c in range(NCH):
                nc.vector.tensor_scalar(Cf[c][:], iota_sb[:],
                                        clu_sb[:, c:c + 1], None,
                                        op0=mybir.AluOpType.is_equal)
                nc.gpsimd.tensor_scalar(Cb[c][:], iota_sb[:],
                                        clu_sb[:, c:c + 1], None,
                                        op0=mybir.AluOpType.is_equal)

            Csh = [pp.tile([128, KPAD], f32, tag=f"Csh{m}", name=f"Csh{m}") for m in range(MCH)]
            Dsh = [pp.tile([128, KPAD], f32, tag=f"Dsh{m}", name=f"Dsh{m}") for m in range(MCH)]
            for m in range(MCH):
                nc.vector.tensor_scalar(Csh[m][:], iota_sb[:],
                                        clush_sb[:, m:m + 1], None,
                                        op0=mybir.AluOpType.is_equal)
                nc.vector.tensor_scalar(Dsh[m][:], Csh[m][:],
                                        sngl_sb[:, m:m + 1], None,
                                        op0=mybir.AluOpType.mult)

            # ---- A column-shard: u8 -> bf16, and masked tanh score M (f32)
            Abf = [pp.tile([128, SH], bf16, tag=f"Abf{c}", name=f"Abf{c}") for c in range(NCH)]
            Mt = [pp.tile([128, SH], f32, tag=f"M{c}", name=f"M{c}") for c in range(NCH)]
            for c in range(NCH):
                a8 = pipe.tile([128, SH], u8, tag="a8", name="a8")
                nc.sync.dma_start(a8[:], acol8[:, c, :])
                nc.gpsimd.tensor_copy(Abf[c][:], a8[:])
                th = pipe.tile([128, SH], f32, tag="th", name="th")
                nc.scalar.activation(th[:], pp_sb[:],
                                     mybir.ActivationFunctionType.Tanh,
                                     bias=q_sb[:, c:c + 1], scale=1.0)
                nc.vector.tensor_tensor(Mt[c][:], th[:], Abf[c][:],
                                        op=mybir.AluOpType.mult)

            # ---- PT-stage (bf16): PT[j_local, k] = sum_i A[i, j] C[i, k]
            PT = [pp.tile([128, KPAD], f32, tag=f"PT{m}", name=f"PT{m}") for m in range(MCH)]
            for m in range(MCH):
                acc = ps.tile([128, KPAD], f32, tag="ptps", name="ptps")
                for c in range(NCH):
                    nc.tensor.matmul(acc[:], Abf[c][:, m * 128:(m + 1) * 128],
                                     Cb[c][:], start=(c == 0), stop=(c == NCH - 1))
                nc.vector.tensor_copy(PT[m][:], acc[:])

            # ---- W-stage (f32): W[i_local, k] = sum_j M[j, i] C[j, k] (+ D)
            Wt = [pp.tile([128, KPAD], f32, tag=f"W{m}", name=f"W{m}") for m in range(MCH)]
            for m in range(MCH):
                acc = ps.tile([128, KPAD], f32, tag="wps", name="wps")
                for c in range(NCH):
                    nc.tensor.matmul(acc[:], Mt[c][:, m * 128:(m + 1) * 128],
                                     Cf[c][:], start=(c == 0), stop=(c == NCH - 1))
                nc.vector.tensor_tensor(Wt[m][:], acc[:], Dsh[m][:],
                                        op=mybir.AluOpType.add)

            # ---- X-stage: X[k, f] = sum_i W[i, k] x[i, f]
            xo_sb = pp.tile([128, 2, F], f32, tag="xout", name="xout")
            for kc in range(2):
                acc = ps.tile([128, F], f32, tag="xps", name="xps")
                for m in range(MCH):
                    nc.tensor.matmul(acc[:], Wt[m][:, kc * 128:(kc + 1) * 128],
                                     x_sb[:, m, :], start=(m == 0), stop=(m == MCH - 1))
                nc.vector.tensor_copy(xo_sb[:, kc, :], acc[:])
                nc.sync.dma_start(xpart[:, kc, :], xo_sb[:, kc, :])

            # ---- An-stage: An[k, l] = sum_j PT[j, k] Csh[j, l]
            an_sb = pp.tile([128, 2, KPAD], f32, tag="anout", name="anout")
            for kc in range(2):
                acc = ps.tile([128, KPAD], f32, tag="anps", name="anps")
                for m in range(MCH):
                    nc.tensor.matmul(acc[:], PT[m][:, kc * 128:(kc + 1) * 128],
                                     Csh[m][:], start=(m == 0), stop=(m == MCH - 1))
                nc.vector.tensor_copy(an_sb[:, kc, :], acc[:])
                nc.sync.dma_start(anpart[:, kc, :], an_sb[:, kc, :])

    nc.compile()
    return nc


def _get_program():
    global _PROG
    if _PROG is None:
        _PROG = _build_program()
    return _PROG


def _core_inputs(r, x, p, q, b, A, cluster, single):
    cols = slice(r * SH, (r + 1) * SH)
    pp_ = (p[cols] + np.float32(b)).astype(np.float32)
    return {
        "acol8": np.ascontiguousarray(
            A[:, cols].reshape(NCH, 128, SH).transpose(1, 0, 2)),
        "xsh": np.ascontiguousarray(
            x[cols].reshape(MCH, 128, F).transpose(1, 0, 2).astype(np.float32)),
        "ppmat": np.ascontiguousarray(
            np.broadcast_to(pp_[None, :], (128, SH)).astype(np.float32)),
        "qmat": np.ascontiguousarray(q.reshape(NCH, 128).T.astype(np.float32)),
        "clumat": np.ascontiguousarray(
            cluster.reshape(NCH, 128).T.astype(np.float32)),
        "clush": np.ascontiguousarray(
            cluster[cols].reshape(MCH, 128).T.astype(np.float32)),
        "snglsh": np.ascontiguousarray(
            single[cols].reshape(MCH, 128).T.astype(np.float32)),
    }


def kernel(x, edge_index, batch, w, b):
    x = np.asarray(x)
    batch = np.asarray(batch)
    p, q, A, cluster, K, single = _host_prep(x, edge_index, batch, w, b)
    if K > KPAD:
        return _numpy_fallback(x, batch, p, q, b, A, cluster, single)

    from concourse.bass_utils import run_bass_kernel_spmd
    nc = _get_program()
    in_maps = [_core_inputs(r, x, p, q, b, A, cluster, single)
               for r in range(NCORES)]
    res = run_bass_kernel_spmd(nc, in_maps, list(range(NCORES))).results

    Xp = np.zeros((KPAD, F), dtype=np.float32)
    Anp = np.zeros((KPAD, KPAD), dtype=np.float32)
    for r in range(NCORES):
        Xp += res[r]["xpart"].transpose(1, 0, 2).reshape(KPAD, F)
        Anp += res[r]["anpart"].transpose(1, 0, 2).reshape(KPAD, KPAD)
    np.fill_diagonal(Anp, 0.0)

    X_new = np.zeros((N, F), dtype=np.float32)
    X_new[:KPAD] = Xp
    A_new = np.zeros((N, N), dtype=np.float32)
    A_new[:KPAD, :KPAD] = Anp
    new_batch = np.zeros(N, dtype=batch.dtype)
    np.maximum.at(new_batch, cluster, batch)
    return X_new, A_new, new_batch, cluster.astype(np.int32)


# revision 8
# speedup vs baseline: 2.0374x; 2.0374x over previous
"""ClusterPooling kernel for 8x Trainium2 NeuronCores (Bass/Tile).

Decomposition (validated against the jax reference):
  e(u,v) = tanh(p_u + q_v + b),  p = x @ w[:F], q = x @ w[F:]
  A   = symmetric 0/1 adjacency (diag 0)        [dense, needed on device]
  S   = A * tanh(p+q+b) outer-sum mask          [dense, device]
  A_c = directed contracted adjacency           [sparse, host: sign tests only]
  labels = reference's directed min-label fixed point  [sparse O(E), host]
  cluster/C = consecutive relabel one-hot       [device builds C from ids]
  X_new = (S@C).T @ x ; A_new = C.T @ A @ C     [dense matmuls, device]

Sharding: core r owns columns R_r = [r*384, (r+1)*384) of the dense
matrices (column shard of A == row shard of A^T; A symmetric).  Each core
computes [256,128] / [256,256] partials; host sums the 8 partials and
embeds into the padded full-size outputs.
"""
import numpy as np

N, F, E = 3072, 128, 24576
NCORES = 8
SH = N // NCORES          # 384 shard columns per core
NCH = N // 128            # 24 partition chunks of the full dim
MCH = SH // 128           # 3 partition chunks of the shard dim
KPAD = 256                # padded cluster count (2 x 128)

_PROG = None              # cached compiled program


# ---------------------------------------------------------------- host math
def _host_prep(x, edge_index, batch, w, b):
    x = np.asarray(x, dtype=np.float32)
    w = np.asarray(w, dtype=np.float32)
    ei = np.asarray(edge_index)
    b = np.float32(np.asarray(b))

    src = ei[0].astype(np.int64)
    dst = ei[1].astype(np.int64)
    s2 = np.concatenate([src, dst])
    d2 = np.concatenate([dst, src])

    # Sign-critical per-edge scores: replicate the reference's exact ops on
    # jax CPU so threshold decisions bit-match the oracle.
    try:
        import jax
        import jax.numpy as jnp
        cpu = jax.devices("cpu")[0]
        with jax.default_device(cpu):
            xj = jnp.asarray(x)
            wj = jnp.asarray(w)
            t = xj[s2] @ wj[:F] + xj[d2] @ wj[F:] + jnp.asarray(b)
            contract = np.asarray(t > 0.0)
    except Exception:
        p_ = x @ w[:F]
        q_ = x @ w[F:]
        contract = (p_[s2] + q_[d2] + float(b)) > 0.0

    p = x @ w[:F]
    q = x @ w[F:]

    # symmetric adjacency mask, diag zero
    A = np.zeros((N, N), dtype=np.uint8)
    A[s2, d2] = 1
    np.fill_diagonal(A, 0)

    keep = contract & (s2 != d2)
    cs, cd = s2[keep], d2[keep]

    # reference's directed min-label propagation + pointer jumping, exactly
    labels = np.arange(N, dtype=np.int64)
    while True:
        neigh = np.full(N, N, dtype=np.int64)
        np.minimum.at(neigh, cs, labels[cd])
        new = np.minimum(labels, neigh)
        new = np.minimum(new, new[new])
        if np.array_equal(new, labels):
            break
        labels = new

    roots = (labels == np.arange(N)).astype(np.int64)
    cluster = (np.cumsum(roots) - 1)[labels]
    K = int(cluster.max()) + 1

    deg = np.zeros(N, dtype=np.int64)
    np.add.at(deg, cs, 1)
    np.add.at(deg, cd, 1)
    single = (deg == 0).astype(np.float32)

    return p, q, A, cluster, K, single


def _numpy_fallback(x, batch, p, q, b, A, cluster, single):
    """Dense-on-host path, used only if K > KPAD (never for the fixed input)."""
    K = int(cluster.max()) + 1
    C = np.zeros((N, K), dtype=np.float32)
    C[np.arange(N), cluster] = 1.0
    Tm = np.tanh((p + float(b))[None, :] + q[:, None]).astype(np.float32)
    M = A.astype(np.float32).T * Tm            # M[j,i] = S[i,j]
    W = M.T @ C + single[:, None] * C
    X = W.T @ x.astype(np.float32)
    An = C.T @ A.astype(np.float32) @ C
    np.fill_diagonal(An, 0.0)
    X_new = np.zeros((N, F), dtype=np.float32)
    X_new[:K] = X
    A_new = np.zeros((N, N), dtype=np.float32)
    A_new[:K, :K] = An
    new_batch = np.zeros(N, dtype=np.asarray(batch).dtype)
    np.maximum.at(new_batch, cluster, np.asarray(batch))
    return X_new, A_new, new_batch, cluster.astype(np.int32)


# ------------------------------------------------------------- bass program
USE_F32R = True      # single-pass fp32 matmuls for W/An stages (4x faster)
NDMA = 4             # acol8 DMA / cast split
NMASK = 6            # mask multiply split


def _build_program():
    import concourse.bacc as bacc
    import concourse.tile as tile
    import concourse.mybir as mybir

    f32 = mybir.dt.float32
    f32r = mybir.dt.float32r
    bf16 = mybir.dt.bfloat16
    u8 = mybir.dt.uint8
    wdt = f32r if USE_F32R else f32

    nc = bacc.Bacc("TRN2", target_bir_lowering=False, debug=False,
                   num_devices=NCORES)

    acol8 = nc.dram_tensor("acol8", [128, NCH * SH], u8, kind="ExternalInput").ap()
    cbf = nc.dram_tensor("cbf", [128, NCH * KPAD], bf16, kind="ExternalInput").ap()
    cshf = nc.dram_tensor("cshf", [128, MCH * KPAD], wdt, kind="ExternalInput").ap()
    dsh = nc.dram_tensor("dsh", [128, MCH, KPAD], f32, kind="ExternalInput").ap()
    xsh = nc.dram_tensor("xsh", [128, MCH, F], f32, kind="ExternalInput").ap()
    ppmat = nc.dram_tensor("ppmat", [128, SH], f32, kind="ExternalInput").ap()
    qmat = nc.dram_tensor("qmat", [128, NCH], f32, kind="ExternalInput").ap()

    xpart = nc.dram_tensor("xpart", [128, 2, F], f32, kind="ExternalOutput").ap()
    anpart = nc.dram_tensor("anpart", [128, 2, KPAD], f32, kind="ExternalOutput").ap()

    AW = NCH * SH          # 9216  free bytes of A per partition
    CW = NCH * KPAD        # 6144  one-hot C width

    with tile.TileContext(nc) as tc:
        with (
            tc.tile_pool(name="persist", bufs=1) as pp,
            tc.tile_pool(name="psum", bufs=2, space="PSUM") as ps,
        ):
            # ---- inputs
            a8_sb = pp.tile([128, AW], u8, tag="a8", name="a8")
            cb_sb = pp.tile([128, CW], bf16, tag="cb", name="cb")
            csh_sb = pp.tile([128, MCH * KPAD], wdt, tag="csh", name="csh")
            dsh_sb = pp.tile([128, MCH, KPAD], f32, tag="dsh", name="dsh")
            x_sb = pp.tile([128, MCH, F], f32, tag="xsh", name="xsh")
            pp_sb = pp.tile([128, SH], f32, tag="ppm", name="ppm")
            q_sb = pp.tile([128, NCH], f32, tag="qm", name="qm")
            g = AW // NDMA
            for i in range(NDMA):
                nc.sync.dma_start(a8_sb[:, i * g:(i + 1) * g],
                                  acol8[:, i * g:(i + 1) * g])
            nc.sync.dma_start(cb_sb[:], cbf[:])
            nc.sync.dma_start(csh_sb[:], cshf[:])
            nc.sync.dma_start(dsh_sb[:], dsh[:])
            nc.sync.dma_start(x_sb[:], xsh[:])
            nc.sync.dma_start(pp_sb[:], ppmat[:])
            nc.sync.dma_start(q_sb[:], qmat[:])

            # ---- cast A u8 -> bf16 (gpsimd, big strips)
            ab_sb = pp.tile([128, AW], bf16, tag="ab", name="ab")
            for i in range(NDMA):
                nc.gpsimd.tensor_copy(ab_sb[:, i * g:(i + 1) * g],
                                      a8_sb[:, i * g:(i + 1) * g])

            # ---- C one-hot in f32 for the 32-bit W-stage matmuls
            cf_sb = pp.tile([128, CW], wdt, tag="cf", name="cf")
            gc = CW // 2
            for i in range(2):
                nc.vector.tensor_copy(cf_sb[:, i * gc:(i + 1) * gc],
                                      cb_sb[:, i * gc:(i + 1) * gc])

            # ---- tanh(p'_i + q_j) per chunk (ACT), then mask (DVE strips)
            th_sb = pp.tile([128, AW], f32, tag="th", name="th")
            for c in range(NCH):
                nc.scalar.activation(th_sb[:, c * SH:(c + 1) * SH], pp_sb[:],
                                     mybir.ActivationFunctionType.Tanh,
                                     bias=q_sb[:, c:c + 1], scale=1.0)
            m_sb = pp.tile([128, AW], wdt, tag="msb", name="msb")
            gm = AW // NMASK
            for i in range(NMASK):
                nc.vector.tensor_tensor(m_sb[:, i * gm:(i + 1) * gm],
                                        th_sb[:, i * gm:(i + 1) * gm],
                                        ab_sb[:, i * gm:(i + 1) * gm],
                                        op=mybir.AluOpType.mult)

            # ---- PT-stage (bf16): PT[j_local, k] = sum_i A[i, j] C[i, k]
            PT = [pp.tile([128, KPAD], wdt, tag=f"PT{m}", name=f"PT{m}")
                  for m in range(MCH)]
            for m in range(MCH):
                acc = ps.tile([128, KPAD], f32, tag="ptps", name="ptps")
                for c in range(NCH):
                    nc.tensor.matmul(acc[:],
                                     ab_sb[:, c * SH + m * 128: c * SH + (m + 1) * 128],
                                     cb_sb[:, c * KPAD:(c + 1) * KPAD],
                                     start=(c == 0), stop=(c == NCH - 1))
                nc.scalar.copy(PT[m][:], acc[:])

            # ---- W-stage: W[i_local, k] = sum_j M[j, i] C[j, k]
            Wt = [pp.tile([128, KPAD], f32, tag=f"W{m}", name=f"W{m}")
                  for m in range(MCH)]
            for m in range(MCH):
                acc = ps.tile([128, KPAD], f32, tag="wps", name="wps")
                for c in range(NCH):
                    nc.tensor.matmul(acc[:],
                                     m_sb[:, c * SH + m * 128: c * SH + (m + 1) * 128],
                                     cf_sb[:, c * KPAD:(c + 1) * KPAD],
                                     start=(c == 0), stop=(c == NCH - 1))
                nc.scalar.copy(Wt[m][:], acc[:])

            # ---- X-stage: X[k, f] = sum_i (W[i,k] + D[i,k]) x[i, f]
            xo_sb = pp.tile([128, 2, F], f32, tag="xout", name="xout")
            for kc in range(2):
                acc = ps.tile([128, F], f32, tag="xps", name="xps")
                for m in range(MCH):
                    nc.tensor.matmul(acc[:], Wt[m][:, kc * 128:(kc + 1) * 128],
                                     x_sb[:, m, :], start=(m == 0), stop=False)
                for m in range(MCH):
                    nc.tensor.matmul(acc[:], dsh_sb[:, m, kc * 128:(kc + 1) * 128],
                                     x_sb[:, m, :], start=False, stop=(m == MCH - 1))
                nc.vector.tensor_copy(xo_sb[:, kc, :], acc[:])
                nc.sync.dma_start(xpart[:, kc, :], xo_sb[:, kc, :])

            # ---- An-stage: An[k, l] = sum_j PT[j, k] Csh[j, l]
            an_sb = pp.tile([128, 2, KPAD], f32, tag="anout", name="anout")
            for kc in range(2):
                acc = ps.tile([128, KPAD], f32, tag="anps", name="anps")
                for m in range(MCH):
                    nc.tensor.matmul(acc[:], PT[m][:, kc * 128:(kc + 1) * 128],
                                     csh_sb[:, m * KPAD:(m + 1) * KPAD],
                                     start=(m == 0), stop=(m == MCH - 1))
                nc.vector.tensor_copy(an_sb[:, kc, :], acc[:])
                nc.sync.dma_start(anpart[:, kc, :], an_sb[:, kc, :])

    nc.compile()
    return nc


def _get_program():
    global _PROG
    if _PROG is None:
        _PROG = _build_program()
    return _PROG


def _core_inputs(r, x, p, q, b, A, cluster, single):
    import ml_dtypes
    bf = ml_dtypes.bfloat16
    cols = slice(r * SH, (r + 1) * SH)
    pp_ = (p[cols] + np.float32(b)).astype(np.float32)
    C = np.zeros((N, KPAD), dtype=bf)
    C[np.arange(N), cluster] = 1
    Csh = C[cols]
    D = single[cols][:, None].astype(np.float32) * Csh.astype(np.float32)
    return {
        "acol8": np.ascontiguousarray(
            A[:, cols].reshape(NCH, 128, SH).transpose(1, 0, 2)
        ).reshape(128, NCH * SH),
        "cbf": np.ascontiguousarray(
            C.reshape(NCH, 128, KPAD).transpose(1, 0, 2)).reshape(128, NCH * KPAD),
        "cshf": np.ascontiguousarray(
            Csh.reshape(MCH, 128, KPAD).transpose(1, 0, 2)
        ).reshape(128, MCH * KPAD).astype(np.float32),
        "dsh": np.ascontiguousarray(
            D.reshape(MCH, 128, KPAD).transpose(1, 0, 2)),
        "xsh": np.ascontiguousarray(
            x[cols].reshape(MCH, 128, F).transpose(1, 0, 2).astype(np.float32)),
        "ppmat": np.ascontiguousarray(
            np.broadcast_to(pp_[None, :], (128, SH)).astype(np.float32)),
        "qmat": np.ascontiguousarray(q.reshape(NCH, 128).T.astype(np.float32)),
    }


def kernel(x, edge_index, batch, w, b):
    x = np.asarray(x)
    batch = np.asarray(batch)
    p, q, A, cluster, K, single = _host_prep(x, edge_index, batch, w, b)
    if K > KPAD:
        return _numpy_fallback(x, batch, p, q, b, A, cluster, single)

    from concourse.bass_utils import run_bass_kernel_spmd
    nc = _get_program()
    in_maps = [_core_inputs(r, x, p, q, b, A, cluster, single)
               for r in range(NCORES)]
    res = run_bass_kernel_spmd(nc, in_maps, list(range(NCORES))).results

    Xp = np.zeros((KPAD, F), dtype=np.float32)
    Anp = np.zeros((KPAD, KPAD), dtype=np.float32)
    for r in range(NCORES):
        Xp += res[r]["xpart"].transpose(1, 0, 2).reshape(KPAD, F)
        Anp += res[r]["anpart"].transpose(1, 0, 2).reshape(KPAD, KPAD)
    np.fill_diagonal(Anp, 0.0)

    X_new = np.zeros((N, F), dtype=np.float32)
    X_new[:KPAD] = Xp
    A_new = np.zeros((N, N), dtype=np.float32)
    A_new[:KPAD, :KPAD] = Anp
    new_batch = np.zeros(N, dtype=batch.dtype)
    np.maximum.at(new_batch, cluster, batch)
    return X_new, A_new, new_batch, cluster.astype(np.int32)


# revision 11
# speedup vs baseline: 3.6809x; 1.8066x over previous
"""ClusterPooling kernel for 8x Trainium2 NeuronCores (Bass/Tile).

Decomposition (validated against the jax reference):
  e(u,v) = tanh(p_u + q_v + b),  p = x @ w[:F], q = x @ w[F:]
  A   = symmetric 0/1 adjacency (diag 0)        [dense, needed on device]
  S   = A * tanh(p+q+b) outer-sum mask          [dense, device]
  A_c = directed contracted adjacency           [sparse, host: sign tests only]
  labels = reference's directed min-label fixed point  [sparse O(E), host]
  cluster/C = consecutive relabel one-hot       [device builds C from ids]
  X_new = (S@C).T @ x ; A_new = C.T @ A @ C     [dense matmuls, device]

Sharding: core r owns columns R_r = [r*384, (r+1)*384) of the dense
matrices (column shard of A == row shard of A^T; A symmetric).  Each core
computes [256,128] / [256,256] partials; host sums the 8 partials and
embeds into the padded full-size outputs.
"""
import numpy as np

N, F, E = 3072, 128, 24576
NCORES = 8
SH = N // NCORES          # 384 shard columns per core
NCH = N // 128            # 24 partition chunks of the full dim
MCH = SH // 128           # 3 partition chunks of the shard dim
KPAD = 256                # padded cluster count (2 x 128)

_PROG = None              # cached compiled program


# ---------------------------------------------------------------- host math
def _host_prep(x, edge_index, batch, w, b):
    x = np.asarray(x, dtype=np.float32)
    w = np.asarray(w, dtype=np.float32)
    ei = np.asarray(edge_index)
    b = np.float32(np.asarray(b))

    src = ei[0].astype(np.int64)
    dst = ei[1].astype(np.int64)
    s2 = np.concatenate([src, dst])
    d2 = np.concatenate([dst, src])

    # Sign-critical per-edge scores: replicate the reference's exact ops on
    # jax CPU so threshold decisions bit-match the oracle.
    try:
        import jax
        import jax.numpy as jnp
        cpu = jax.devices("cpu")[0]
        with jax.default_device(cpu):
            xj = jnp.asarray(x)
            wj = jnp.asarray(w)
            t = xj[s2] @ wj[:F] + xj[d2] @ wj[F:] + jnp.asarray(b)
            contract = np.asarray(t > 0.0)
    except Exception:
        p_ = x @ w[:F]
        q_ = x @ w[F:]
        contract = (p_[s2] + q_[d2] + float(b)) > 0.0

    p = x @ w[:F]
    q = x @ w[F:]

    # symmetric adjacency mask, diag zero
    A = np.zeros((N, N), dtype=np.uint8)
    A[s2, d2] = 1
    np.fill_diagonal(A, 0)

    keep = contract & (s2 != d2)
    cs, cd = s2[keep], d2[keep]

    # reference's directed min-label propagation + pointer jumping, exactly
    labels = np.arange(N, dtype=np.int64)
    while True:
        neigh = np.full(N, N, dtype=np.int64)
        np.minimum.at(neigh, cs, labels[cd])
        new = np.minimum(labels, neigh)
        new = np.minimum(new, new[new])
        if np.array_equal(new, labels):
            break
        labels = new

    roots = (labels == np.arange(N)).astype(np.int64)
    cluster = (np.cumsum(roots) - 1)[labels]
    K = int(cluster.max()) + 1

    deg = np.zeros(N, dtype=np.int64)
    np.add.at(deg, cs, 1)
    np.add.at(deg, cd, 1)
    single = (deg == 0).astype(np.float32)

    return p, q, A, cluster, K, single


def _numpy_fallback(x, batch, p, q, b, A, cluster, single):
    """Dense-on-host path, used only if K > KPAD (never for the fixed input)."""
    K = int(cluster.max()) + 1
    C = np.zeros((N, K), dtype=np.float32)
    C[np.arange(N), cluster] = 1.0
    Tm = np.tanh((p + float(b))[None, :] + q[:, None]).astype(np.float32)
    M = A.astype(np.float32).T * Tm            # M[j,i] = S[i,j]
    W = M.T @ C + single[:, None] * C
    X = W.T @ x.astype(np.float32)
    An = C.T @ A.astype(np.float32) @ C
    np.fill_diagonal(An, 0.0)
    X_new = np.zeros((N, F), dtype=np.float32)
    X_new[:K] = X
    A_new = np.zeros((N, N), dtype=np.float32)
    A_new[:K, :K] = An
    new_batch = np.zeros(N, dtype=np.asarray(batch).dtype)
    np.maximum.at(new_batch, cluster, np.asarray(batch))
    return X_new, A_new, new_batch, cluster.astype(np.int32)


# ------------------------------------------------------------- bass program
USE_F32R = True      # single-pass fp32 matmuls for W/An stages (4x faster)
NDMA = 4             # acol8 DMA / cast split
NMASK = 6            # mask multiply split


def _build_program():
    import concourse.bacc as bacc
    import concourse.tile as tile
    import concourse.mybir as mybir

    f32 = mybir.dt.float32
    f32r = mybir.dt.float32r
    bf16 = mybir.dt.bfloat16
    u8 = mybir.dt.uint8
    wdt = f32r if USE_F32R else f32

    nc = bacc.Bacc("TRN2", target_bir_lowering=False, debug=False,
                   num_devices=NCORES)

    abf = nc.dram_tensor("abf", [128, NCH * SH], bf16, kind="ExternalInput").ap()
    cbf = nc.dram_tensor("cbf", [128, NCH * KPAD], bf16, kind="ExternalInput").ap()
    cf32 = nc.dram_tensor("cf32", [128, NCH * KPAD], wdt, kind="ExternalInput").ap()
    cshf = nc.dram_tensor("cshf", [128, MCH * KPAD], wdt, kind="ExternalInput").ap()
    dsh = nc.dram_tensor("dsh", [128, MCH, KPAD], wdt, kind="ExternalInput").ap()
    xsh = nc.dram_tensor("xsh", [128, MCH, F], wdt, kind="ExternalInput").ap()
    ppmat = nc.dram_tensor("ppmat", [128, SH], f32, kind="ExternalInput").ap()
    qmat = nc.dram_tensor("qmat", [128, NCH], f32, kind="ExternalInput").ap()

    xpart = nc.dram_tensor("xpart", [128, KPAD], f32, kind="ExternalOutput").ap()
    anpart = nc.dram_tensor("anpart", [128, 2, KPAD], f32, kind="ExternalOutput").ap()

    AW = NCH * SH          # 9216  A columns per partition
    CW = NCH * KPAD        # 6144  one-hot C width

    with tile.TileContext(nc) as tc:
        with (
            tc.tile_pool(name="persist", bufs=1) as pp,
            tc.tile_pool(name="psum", bufs=2, space="PSUM") as ps,
        ):
            # ---- inputs
            ab_sb = pp.tile([128, AW], bf16, tag="ab", name="ab")
            cb_sb = pp.tile([128, CW], bf16, tag="cb", name="cb")
            cf_sb = pp.tile([128, CW], wdt, tag="cf", name="cf")
            csh_sb = pp.tile([128, MCH * KPAD], wdt, tag="csh", name="csh")
            dsh_sb = pp.tile([128, MCH, KPAD], wdt, tag="dsh", name="dsh")
            x_sb = pp.tile([128, MCH, F], wdt, tag="xsh", name="xsh")
            pp_sb = pp.tile([128, SH], f32, tag="ppm", name="ppm")
            q_sb = pp.tile([128, NCH], f32, tag="qm", name="qm")
            nc.sync.dma_start(pp_sb[:], ppmat[:])
            nc.sync.dma_start(q_sb[:], qmat[:])
            g = AW // NDMA
            for i in range(NDMA):
                nc.sync.dma_start(ab_sb[:, i * g:(i + 1) * g],
                                  abf[:, i * g:(i + 1) * g])
            nc.sync.dma_start(cb_sb[:], cbf[:])
            gc = CW // 2
            for i in range(2):
                nc.sync.dma_start(cf_sb[:, i * gc:(i + 1) * gc],
                                  cf32[:, i * gc:(i + 1) * gc])
            nc.sync.dma_start(csh_sb[:], cshf[:])
            nc.sync.dma_start(dsh_sb[:], dsh[:])
            nc.sync.dma_start(x_sb[:], xsh[:])

            # ---- tanh(p'_i + q_j) per chunk (ACT), then mask (DVE strips)
            th_sb = pp.tile([128, AW], f32, tag="th", name="th")
            for c in range(NCH):
                nc.scalar.activation(th_sb[:, c * SH:(c + 1) * SH], pp_sb[:],
                                     mybir.ActivationFunctionType.Tanh,
                                     bias=q_sb[:, c:c + 1], scale=1.0)
            m_sb = pp.tile([128, AW], wdt, tag="msb", name="msb")
            gm = AW // NMASK
            for i in range(NMASK):
                nc.vector.tensor_tensor(m_sb[:, i * gm:(i + 1) * gm],
                                        th_sb[:, i * gm:(i + 1) * gm],
                                        ab_sb[:, i * gm:(i + 1) * gm],
                                        op=mybir.AluOpType.mult)

            # ---- PT-stage (bf16): PT[j_local, k] = sum_i A[i, j] C[i, k]
            PT = [pp.tile([128, KPAD], wdt, tag=f"PT{m}", name=f"PT{m}")
                  for m in range(MCH)]
            for m in range(MCH):
                acc = ps.tile([128, KPAD], f32, tag="ptps", name="ptps")
                for c in range(NCH):
                    nc.tensor.matmul(acc[:],
                                     ab_sb[:, c * SH + m * 128: c * SH + (m + 1) * 128],
                                     cb_sb[:, c * KPAD:(c + 1) * KPAD],
                                     start=(c == 0), stop=(c == NCH - 1))
                nc.scalar.copy(PT[m][:], acc[:])

            # ---- W-stage (f32r): W[i_local, k] = sum_j M[j, i] C[j, k]
            Wt = [pp.tile([128, KPAD], wdt, tag=f"W{m}", name=f"W{m}")
                  for m in range(MCH)]
            for m in range(MCH):
                acc = ps.tile([128, KPAD], f32, tag="wps", name="wps")
                for c in range(NCH):
                    nc.tensor.matmul(acc[:],
                                     m_sb[:, c * SH + m * 128: c * SH + (m + 1) * 128],
                                     cf_sb[:, c * KPAD:(c + 1) * KPAD],
                                     start=(c == 0), stop=(c == NCH - 1))
                nc.scalar.copy(Wt[m][:], acc[:])

            # ---- X-stage (transposed, f32r): X.T[f, k] = sum_i x[i,f](W+D)[i,k]
            xo_sb = pp.tile([128, KPAD], f32, tag="xout", name="xout")
            acc = ps.tile([128, KPAD], f32, tag="xps", name="xps")
            for m in range(MCH):
                nc.tensor.matmul(acc[:], x_sb[:, m, :], Wt[m][:],
                                 start=(m == 0), stop=False)
            for m in range(MCH):
                nc.tensor.matmul(acc[:], x_sb[:, m, :], dsh_sb[:, m, :],
                                 start=False, stop=(m == MCH - 1))
            nc.vector.tensor_copy(xo_sb[:], acc[:])
            nc.sync.dma_start(xpart[:], xo_sb[:])

            # ---- An-stage (f32r): An[k, l] = sum_j PT[j, k] Csh[j, l]
            an_sb = pp.tile([128, 2, KPAD], f32, tag="anout", name="anout")
            for kc in range(2):
                acc2 = ps.tile([128, KPAD], f32, tag="anps", name="anps")
                for m in range(MCH):
                    nc.tensor.matmul(acc2[:], PT[m][:, kc * 128:(kc + 1) * 128],
                                     csh_sb[:, m * KPAD:(m + 1) * KPAD],
                                     start=(m == 0), stop=(m == MCH - 1))
                nc.vector.tensor_copy(an_sb[:, kc, :], acc2[:])
                nc.sync.dma_start(anpart[:, kc, :], an_sb[:, kc, :])

    nc.compile()
    return nc


def _get_program():
    global _PROG
    if _PROG is None:
        _PROG = _build_program()
    return _PROG


def _core_inputs(r, x, p, q, b, A, cluster, single):
    import ml_dtypes
    bf = ml_dtypes.bfloat16
    cols = slice(r * SH, (r + 1) * SH)
    pp_ = (p[cols] + np.float32(b)).astype(np.float32)
    C = np.zeros((N, KPAD), dtype=bf)
    C[np.arange(N), cluster] = 1
    Csh = C[cols]
    D = single[cols][:, None].astype(np.float32) * Csh.astype(np.float32)
    return {
        "abf": np.ascontiguousarray(
            A[:, cols].reshape(NCH, 128, SH).transpose(1, 0, 2)
        ).reshape(128, NCH * SH).astype(bf),
        "cbf": np.ascontiguousarray(
            C.reshape(NCH, 128, KPAD).transpose(1, 0, 2)).reshape(128, NCH * KPAD),
        "cf32": np.ascontiguousarray(
            C.reshape(NCH, 128, KPAD).transpose(1, 0, 2)
        ).reshape(128, NCH * KPAD).astype(np.float32),
        "cshf": np.ascontiguousarray(
            Csh.reshape(MCH, 128, KPAD).transpose(1, 0, 2)
        ).reshape(128, MCH * KPAD).astype(np.float32),
        "dsh": np.ascontiguousarray(
            D.reshape(MCH, 128, KPAD).transpose(1, 0, 2)),
        "xsh": np.ascontiguousarray(
            x[cols].reshape(MCH, 128, F).transpose(1, 0, 2).astype(np.float32)),
        "ppmat": np.ascontiguousarray(
            np.broadcast_to(pp_[None, :], (128, SH)).astype(np.float32)),
        "qmat": np.ascontiguousarray(q.reshape(NCH, 128).T.astype(np.float32)),
    }


def kernel(x, edge_index, batch, w, b):
    x = np.asarray(x)
    batch = np.asarray(batch)
    p, q, A, cluster, K, single = _host_prep(x, edge_index, batch, w, b)
    if K > KPAD:
        return _numpy_fallback(x, batch, p, q, b, A, cluster, single)

    from concourse.bass_utils import run_bass_kernel_spmd
    nc = _get_program()
    in_maps = [_core_inputs(r, x, p, q, b, A, cluster, single)
               for r in range(NCORES)]
    res = run_bass_kernel_spmd(nc, in_maps, list(range(NCORES))).results

    Xp = np.zeros((KPAD, F), dtype=np.float32)
    Anp = np.zeros((KPAD, KPAD), dtype=np.float32)
    for r in range(NCORES):
        Xp += res[r]["xpart"].T
        Anp += res[r]["anpart"].transpose(1, 0, 2).reshape(KPAD, KPAD)
    np.fill_diagonal(Anp, 0.0)

    X_new = np.zeros((N, F), dtype=np.float32)
    X_new[:KPAD] = Xp
    A_new = np.zeros((N, N), dtype=np.float32)
    A_new[:KPAD, :KPAD] = Anp
    new_batch = np.zeros(N, dtype=batch.dtype)
    np.maximum.at(new_batch, cluster, batch)
    return X_new, A_new, new_batch, cluster.astype(np.int32)


# revision 13
# speedup vs baseline: 3.7395x; 1.0159x over previous
"""ClusterPooling kernel for 8x Trainium2 NeuronCores (Bass/Tile).

Decomposition (validated against the jax reference):
  e(u,v) = tanh(p_u + q_v + b),  p = x @ w[:F], q = x @ w[F:]
  A   = symmetric 0/1 adjacency (diag 0)        [dense, needed on device]
  S   = A * tanh(p+q+b) outer-sum mask          [dense, device]
  A_c = directed contracted adjacency           [sparse, host: sign tests only]
  labels = reference's directed min-label fixed point  [sparse O(E), host]
  cluster/C = consecutive relabel one-hot       [device builds C from ids]
  X_new = (S@C).T @ x ; A_new = C.T @ A @ C     [dense matmuls, device]

Sharding: core r owns columns R_r = [r*384, (r+1)*384) of the dense
matrices (column shard of A == row shard of A^T; A symmetric).  Each core
computes [256,128] / [256,256] partials; host sums the 8 partials and
embeds into the padded full-size outputs.
"""
import numpy as np

N, F, E = 3072, 128, 24576
NCORES = 8
SH = N // NCORES          # 384 shard columns per core
NCH = N // 128            # 24 partition chunks of the full dim
MCH = SH // 128           # 3 partition chunks of the shard dim
KPAD = 256                # padded cluster count (2 x 128)

_PROG = None              # cached compiled program


# ---------------------------------------------------------------- host math
def _host_prep(x, edge_index, batch, w, b):
    x = np.asarray(x, dtype=np.float32)
    w = np.asarray(w, dtype=np.float32)
    ei = np.asarray(edge_index)
    b = np.float32(np.asarray(b))

    src = ei[0].astype(np.int64)
    dst = ei[1].astype(np.int64)
    s2 = np.concatenate([src, dst])
    d2 = np.concatenate([dst, src])

    # Sign-critical per-edge scores: replicate the reference's exact ops on
    # jax CPU so threshold decisions bit-match the oracle.
    try:
        import jax
        import jax.numpy as jnp
        cpu = jax.devices("cpu")[0]
        with jax.default_device(cpu):
            xj = jnp.asarray(x)
            wj = jnp.asarray(w)
            t = xj[s2] @ wj[:F] + xj[d2] @ wj[F:] + jnp.asarray(b)
            contract = np.asarray(t > 0.0)
    except Exception:
        p_ = x @ w[:F]
        q_ = x @ w[F:]
        contract = (p_[s2] + q_[d2] + float(b)) > 0.0

    p = x @ w[:F]
    q = x @ w[F:]

    # symmetric adjacency mask, diag zero
    A = np.zeros((N, N), dtype=np.uint8)
    A[s2, d2] = 1
    np.fill_diagonal(A, 0)

    keep = contract & (s2 != d2)
    cs, cd = s2[keep], d2[keep]

    # reference's directed min-label propagation + pointer jumping, exactly
    labels = np.arange(N, dtype=np.int64)
    while True:
        neigh = np.full(N, N, dtype=np.int64)
        np.minimum.at(neigh, cs, labels[cd])
        new = np.minimum(labels, neigh)
        new = np.minimum(new, new[new])
        if np.array_equal(new, labels):
            break
        labels = new

    roots = (labels == np.arange(N)).astype(np.int64)
    cluster = (np.cumsum(roots) - 1)[labels]
    K = int(cluster.max()) + 1

    deg = np.zeros(N, dtype=np.int64)
    np.add.at(deg, cs, 1)
    np.add.at(deg, cd, 1)
    single = (deg == 0).astype(np.float32)

    return p, q, A, cluster, K, single


def _numpy_fallback(x, batch, p, q, b, A, cluster, single):
    """Dense-on-host path, used only if K > KPAD (never for the fixed input)."""
    K = int(cluster.max()) + 1
    C = np.zeros((N, K), dtype=np.float32)
    C[np.arange(N), cluster] = 1.0
    Tm = np.tanh((p + float(b))[None, :] + q[:, None]).astype(np.float32)
    M = A.astype(np.float32).T * Tm            # M[j,i] = S[i,j]
    W = M.T @ C + single[:, None] * C
    X = W.T @ x.astype(np.float32)
    An = C.T @ A.astype(np.float32) @ C
    np.fill_diagonal(An, 0.0)
    X_new = np.zeros((N, F), dtype=np.float32)
    X_new[:K] = X
    A_new = np.zeros((N, N), dtype=np.float32)
    A_new[:K, :K] = An
    new_batch = np.zeros(N, dtype=np.asarray(batch).dtype)
    np.maximum.at(new_batch, cluster, np.asarray(batch))
    return X_new, A_new, new_batch, cluster.astype(np.int32)


# ------------------------------------------------------------- bass program
USE_F32R = True      # single-pass fp32 matmuls for W/An stages (4x faster)
NDMA = 4             # acol8 DMA / cast split
NMASK = 6            # mask multiply split


def _build_program():
    import concourse.bacc as bacc
    import concourse.tile as tile
    import concourse.mybir as mybir

    f32 = mybir.dt.float32
    f32r = mybir.dt.float32r
    bf16 = mybir.dt.bfloat16
    f16 = mybir.dt.float16
    wdt = f32r if USE_F32R else f32

    nc = bacc.Bacc("TRN2", target_bir_lowering=False, debug=False,
                   num_devices=NCORES)

    abf = nc.dram_tensor("abf", [128, NCH * SH], bf16, kind="ExternalInput").ap()
    cbf = nc.dram_tensor("cbf", [128, NCH * KPAD], bf16, kind="ExternalInput").ap()
    cshf = nc.dram_tensor("cshf", [128, MCH * KPAD], wdt, kind="ExternalInput").ap()
    xsh = nc.dram_tensor("xsh", [128, MCH, F], wdt, kind="ExternalInput").ap()
    ppmat = nc.dram_tensor("ppmat", [128, SH], f32, kind="ExternalInput").ap()
    qmat = nc.dram_tensor("qmat", [128, NCH], f32, kind="ExternalInput").ap()

    xpart = nc.dram_tensor("xpart", [128, KPAD], f32, kind="ExternalOutput").ap()
    anpart = nc.dram_tensor("anpart", [128, 2, KPAD], f32, kind="ExternalOutput").ap()

    AW = NCH * SH          # 9216  A columns per partition
    CW = NCH * KPAD        # 6144  one-hot C width

    with tile.TileContext(nc) as tc:
        with (
            tc.tile_pool(name="persist", bufs=1) as pp,
            tc.tile_pool(name="psum", bufs=2, space="PSUM") as ps,
        ):
            # ---- inputs (small/early first: tanh can start immediately)
            pp_sb = pp.tile([128, SH], f32, tag="ppm", name="ppm")
            q_sb = pp.tile([128, NCH], f32, tag="qm", name="qm")
            a_sb = pp.tile([128, AW], bf16, tag="ab", name="ab")
            c_sb = pp.tile([128, CW], bf16, tag="cb", name="cb")
            cf_sb = pp.tile([128, CW], wdt, tag="cf", name="cf")
            csh_sb = pp.tile([128, MCH * KPAD], wdt, tag="csh", name="csh")
            x_sb = pp.tile([128, MCH, F], wdt, tag="xsh", name="xsh")
            nc.sync.dma_start(pp_sb[:], ppmat[:])
            nc.sync.dma_start(q_sb[:], qmat[:])
            ga = AW // NMASK
            for i in range(NMASK):
                nc.sync.dma_start(a_sb[:, i * ga:(i + 1) * ga],
                                  abf[:, i * ga:(i + 1) * ga])
            gc = CW // 2
            for i in range(2):
                nc.sync.dma_start(c_sb[:, i * gc:(i + 1) * gc],
                                  cbf[:, i * gc:(i + 1) * gc])
            nc.sync.dma_start(csh_sb[:], cshf[:])
            nc.sync.dma_start(x_sb[:], xsh[:])

            # ---- C one-hot cast bf16 -> f32r for the W-stage (DVE)
            for i in range(4):
                gq = CW // 4
                nc.vector.tensor_copy(cf_sb[:, i * gq:(i + 1) * gq],
                                      c_sb[:, i * gq:(i + 1) * gq])

            # ---- tanh(p'_i + q_j) per chunk (ACT), then mask (DVE strips)
            th_sb = pp.tile([128, AW], f32, tag="th", name="th")
            for c in range(NCH):
                nc.scalar.activation(th_sb[:, c * SH:(c + 1) * SH], pp_sb[:],
                                     mybir.ActivationFunctionType.Tanh,
                                     bias=q_sb[:, c:c + 1], scale=1.0)
            m_sb = pp.tile([128, AW], wdt, tag="msb", name="msb")
            for i in range(NMASK):
                nc.vector.tensor_tensor(m_sb[:, i * ga:(i + 1) * ga],
                                        th_sb[:, i * ga:(i + 1) * ga],
                                        a_sb[:, i * ga:(i + 1) * ga],
                                        op=mybir.AluOpType.mult)

            # ---- PT-stage (fp16): PT[j_local, k] = sum_i A[i, j] C[i, k]
            PT = [pp.tile([128, KPAD], wdt, tag=f"PT{m}", name=f"PT{m}")
                  for m in range(MCH)]
            for m in range(MCH):
                acc = ps.tile([128, KPAD], f32, tag="ptps", name="ptps")
                for c in range(NCH):
                    nc.tensor.matmul(acc[:],
                                     a_sb[:, c * SH + m * 128: c * SH + (m + 1) * 128],
                                     c_sb[:, c * KPAD:(c + 1) * KPAD],
                                     start=(c == 0), stop=(c == NCH - 1))
                nc.scalar.copy(PT[m][:], acc[:])

            # ---- W-stage (fp16): W[i_local, k] = sum_j M[j, i] C[j, k]
            Wt = [pp.tile([128, KPAD], wdt, tag=f"W{m}", name=f"W{m}")
                  for m in range(MCH)]
            for m in range(MCH):
                acc = ps.tile([128, KPAD], f32, tag="wps", name="wps")
                for c in range(NCH):
                    nc.tensor.matmul(acc[:],
                                     m_sb[:, c * SH + m * 128: c * SH + (m + 1) * 128],
                                     cf_sb[:, c * KPAD:(c + 1) * KPAD],
                                     start=(c == 0), stop=(c == NCH - 1))
                nc.scalar.copy(Wt[m][:], acc[:])

            # ---- X-stage (transposed, f32r): X.T[f, k] = sum_i x[i,f] W[i,k]
            xo_sb = pp.tile([128, KPAD], f32, tag="xout", name="xout")
            acc = ps.tile([128, KPAD], f32, tag="xps", name="xps")
            for m in range(MCH):
                nc.tensor.matmul(acc[:], x_sb[:, m, :], Wt[m][:],
                                 start=(m == 0), stop=(m == MCH - 1))
            nc.vector.tensor_copy(xo_sb[:], acc[:])
            nc.sync.dma_start(xpart[:], xo_sb[:])

            # ---- An-stage (fp16): An[k, l] = sum_j PT[j, k] Csh[j, l]
            an_sb = pp.tile([128, 2, KPAD], f32, tag="anout", name="anout")
            for kc in range(2):
                acc2 = ps.tile([128, KPAD], f32, tag="anps", name="anps")
                for m in range(MCH):
                    nc.tensor.matmul(acc2[:], PT[m][:, kc * 128:(kc + 1) * 128],
                                     csh_sb[:, m * KPAD:(m + 1) * KPAD],
                                     start=(m == 0), stop=(m == MCH - 1))
                nc.vector.tensor_copy(an_sb[:, kc, :], acc2[:])
                nc.sync.dma_start(anpart[:, kc, :], an_sb[:, kc, :])

    nc.compile()
    return nc


def _get_program():
    global _PROG
    if _PROG is None:
        _PROG = _build_program()
    return _PROG


def _core_inputs(r, x, p, q, b, A, cluster, single):
    import ml_dtypes
    bf = ml_dtypes.bfloat16
    cols = slice(r * SH, (r + 1) * SH)
    pp_ = (p[cols] + np.float32(b)).astype(np.float32)
    C = np.zeros((N, KPAD), dtype=bf)
    C[np.arange(N), cluster] = 1
    return {
        "abf": np.ascontiguousarray(
            A[:, cols].reshape(NCH, 128, SH).transpose(1, 0, 2)
        ).reshape(128, NCH * SH).astype(bf),
        "cbf": np.ascontiguousarray(
            C.reshape(NCH, 128, KPAD).transpose(1, 0, 2)).reshape(128, NCH * KPAD),
        "cshf": np.ascontiguousarray(
            C[cols].reshape(MCH, 128, KPAD).transpose(1, 0, 2)
        ).reshape(128, MCH * KPAD).astype(np.float32),
        "xsh": np.ascontiguousarray(
            x[cols].reshape(MCH, 128, F).transpose(1, 0, 2).astype(np.float32)),
        "ppmat": np.ascontiguousarray(
            np.broadcast_to(pp_[None, :], (128, SH)).astype(np.float32)),
        "qmat": np.ascontiguousarray(q.reshape(NCH, 128).T.astype(np.float32)),
    }


def kernel(x, edge_index, batch, w, b):
    x = np.asarray(x)
    batch = np.asarray(batch)
    p, q, A, cluster, K, single = _host_prep(x, edge_index, batch, w, b)
    if K > KPAD:
        return _numpy_fallback(x, batch, p, q, b, A, cluster, single)

    from concourse.bass_utils import run_bass_kernel_spmd
    nc = _get_program()
    in_maps = [_core_inputs(r, x, p, q, b, A, cluster, single)
               for r in range(NCORES)]
    res = run_bass_kernel_spmd(nc, in_maps, list(range(NCORES))).results

    Xp = np.zeros((KPAD, F), dtype=np.float32)
    Anp = np.zeros((KPAD, KPAD), dtype=np.float32)
    for r in range(NCORES):
        Xp += res[r]["xpart"].T
        Anp += res[r]["anpart"].transpose(1, 0, 2).reshape(KPAD, KPAD)
    np.fill_diagonal(Anp, 0.0)

    sidx = np.nonzero(single)[0]
    np.add.at(Xp, cluster[sidx], x[sidx].astype(np.float32))
    X_new = np.zeros((N, F), dtype=np.float32)
    X_new[:KPAD] = Xp
    A_new = np.zeros((N, N), dtype=np.float32)
    A_new[:KPAD, :KPAD] = Anp
    new_batch = np.zeros(N, dtype=batch.dtype)
    np.maximum.at(new_batch, cluster, batch)
    return X_new, A_new, new_batch, cluster.astype(np.int32)


# revision 15
# speedup vs baseline: 3.7917x; 1.0140x over previous
"""ClusterPooling kernel for 8x Trainium2 NeuronCores (Bass/Tile).

Decomposition (validated against the jax reference):
  e(u,v) = tanh(p_u + q_v + b),  p = x @ w[:F], q = x @ w[F:]
  A   = symmetric 0/1 adjacency (diag 0)        [dense, needed on device]
  S   = A * tanh(p+q+b) outer-sum mask          [dense, device]
  A_c = directed contracted adjacency           [sparse, host: sign tests only]
  labels = reference's directed min-label fixed point  [sparse O(E), host]
  cluster/C = consecutive relabel one-hot       [device builds C from ids]
  X_new = (S@C).T @ x ; A_new = C.T @ A @ C     [dense matmuls, device]

Sharding: core r owns columns R_r = [r*384, (r+1)*384) of the dense
matrices (column shard of A == row shard of A^T; A symmetric).  Each core
computes [256,128] / [256,256] partials; host sums the 8 partials and
embeds into the padded full-size outputs.
"""
import numpy as np

N, F, E = 3072, 128, 24576
NCORES = 8
SH = N // NCORES          # 384 shard columns per core
NCH = N // 128            # 24 partition chunks of the full dim
MCH = SH // 128           # 3 partition chunks of the shard dim
KPAD = 256                # padded cluster count (2 x 128)

_PROG = None              # cached compiled program


# ---------------------------------------------------------------- host math
def _host_prep(x, edge_index, batch, w, b):
    x = np.asarray(x, dtype=np.float32)
    w = np.asarray(w, dtype=np.float32)
    ei = np.asarray(edge_index)
    b = np.float32(np.asarray(b))

    src = ei[0].astype(np.int64)
    dst = ei[1].astype(np.int64)
    s2 = np.concatenate([src, dst])
    d2 = np.concatenate([dst, src])

    # Sign-critical per-edge scores: replicate the reference's exact ops on
    # jax CPU so threshold decisions bit-match the oracle.
    try:
        import jax
        import jax.numpy as jnp
        cpu = jax.devices("cpu")[0]
        with jax.default_device(cpu):
            xj = jnp.asarray(x)
            wj = jnp.asarray(w)
            t = xj[s2] @ wj[:F] + xj[d2] @ wj[F:] + jnp.asarray(b)
            contract = np.asarray(t > 0.0)
    except Exception:
        p_ = x @ w[:F]
        q_ = x @ w[F:]
        contract = (p_[s2] + q_[d2] + float(b)) > 0.0

    p = x @ w[:F]
    q = x @ w[F:]

    # symmetric adjacency mask, diag zero
    A = np.zeros((N, N), dtype=np.uint8)
    A[s2, d2] = 1
    np.fill_diagonal(A, 0)

    keep = contract & (s2 != d2)
    cs, cd = s2[keep], d2[keep]

    # reference's directed min-label propagation + pointer jumping, exactly
    labels = np.arange(N, dtype=np.int64)
    while True:
        neigh = np.full(N, N, dtype=np.int64)
        np.minimum.at(neigh, cs, labels[cd])
        new = np.minimum(labels, neigh)
        new = np.minimum(new, new[new])
        if np.array_equal(new, labels):
            break
        labels = new

    roots = (labels == np.arange(N)).astype(np.int64)
    cluster = (np.cumsum(roots) - 1)[labels]
    K = int(cluster.max()) + 1

    deg = np.zeros(N, dtype=np.int64)
    np.add.at(deg, cs, 1)
    np.add.at(deg, cd, 1)
    single = (deg == 0).astype(np.float32)

    return p, q, A, cluster, K, single


def _numpy_fallback(x, batch, p, q, b, A, cluster, single):
    """Dense-on-host path, used only if K > KPAD (never for the fixed input)."""
    K = int(cluster.max()) + 1
    C = np.zeros((N, K), dtype=np.float32)
    C[np.arange(N), cluster] = 1.0
    Tm = np.tanh((p + float(b))[None, :] + q[:, None]).astype(np.float32)
    M = A.astype(np.float32).T * Tm            # M[j,i] = S[i,j]
    W = M.T @ C + single[:, None] * C
    X = W.T @ x.astype(np.float32)
    An = C.T @ A.astype(np.float32) @ C
    np.fill_diagonal(An, 0.0)
    X_new = np.zeros((N, F), dtype=np.float32)
    X_new[:K] = X
    A_new = np.zeros((N, N), dtype=np.float32)
    A_new[:K, :K] = An
    new_batch = np.zeros(N, dtype=np.asarray(batch).dtype)
    np.maximum.at(new_batch, cluster, np.asarray(batch))
    return X_new, A_new, new_batch, cluster.astype(np.int32)


# ------------------------------------------------------------- bass program
USE_F32R = True      # single-pass fp32 matmuls for W/An stages (4x faster)
NDMA = 4             # acol8 DMA / cast split
NMASK = 6            # mask multiply split


def _build_program():
    import concourse.bacc as bacc
    import concourse.tile as tile
    import concourse.mybir as mybir

    f32 = mybir.dt.float32
    f32r = mybir.dt.float32r
    bf16 = mybir.dt.bfloat16
    f16 = mybir.dt.float16
    wdt = f32r if USE_F32R else f32

    nc = bacc.Bacc("TRN2", target_bir_lowering=False, debug=False,
                   num_devices=NCORES)

    abf = nc.dram_tensor("abf", [128, NCH * SH], bf16, kind="ExternalInput").ap()
    cbf = nc.dram_tensor("cbf", [128, NCH * KPAD], bf16, kind="ExternalInput").ap()
    cshf = nc.dram_tensor("cshf", [128, MCH * KPAD], wdt, kind="ExternalInput").ap()
    xsh = nc.dram_tensor("xsh", [128, MCH, F], wdt, kind="ExternalInput").ap()
    ppmat = nc.dram_tensor("ppmat", [128, SH], f32, kind="ExternalInput").ap()
    qmat = nc.dram_tensor("qmat", [128, NCH], f32, kind="ExternalInput").ap()

    xpart = nc.dram_tensor("xpart", [128, KPAD], f32, kind="ExternalOutput").ap()
    anpart = nc.dram_tensor("anpart", [128, 2, KPAD], f32, kind="ExternalOutput").ap()

    AW = NCH * SH          # 9216  A columns per partition
    CW = NCH * KPAD        # 6144  one-hot C width

    with tile.TileContext(nc) as tc:
        with (
            tc.tile_pool(name="persist", bufs=1) as pp,
            tc.tile_pool(name="psum", bufs=2, space="PSUM") as ps,
        ):
            # ---- inputs (small/early first: tanh can start immediately)
            # strips are separate tiles so consumers dep only on their strip
            pp_sb = pp.tile([128, SH], f32, tag="ppm", name="ppm")
            q_sb = pp.tile([128, NCH], f32, tag="qm", name="qm")
            ga = AW // NMASK
            gc = CW // 2
            a_t = [pp.tile([128, ga], bf16, tag=f"at{i}", name=f"at{i}")
                   for i in range(NMASK)]
            c_t = [pp.tile([128, gc], bf16, tag=f"ct{i}", name=f"ct{i}")
                   for i in range(2)]
            cf_t = [pp.tile([128, gc], wdt, tag=f"cft{i}", name=f"cft{i}")
                    for i in range(2)]
            csh_sb = pp.tile([128, MCH * KPAD], wdt, tag="csh", name="csh")
            x_sb = pp.tile([128, MCH, F], wdt, tag="xsh", name="xsh")
            # DMA order tuned so PT/W consumption can chase arrivals
            nc.sync.dma_start(pp_sb[:], ppmat[:])
            nc.sync.dma_start(q_sb[:], qmat[:])
            nc.sync.dma_start(c_t[0][:], cbf[:, 0:gc])
            nc.sync.dma_start(a_t[0][:], abf[:, 0:ga])
            nc.sync.dma_start(a_t[1][:], abf[:, ga:2 * ga])
            nc.sync.dma_start(c_t[1][:], cbf[:, gc:])
            for i in range(2, NMASK):
                nc.sync.dma_start(a_t[i][:], abf[:, i * ga:(i + 1) * ga])
            nc.sync.dma_start(csh_sb[:], cshf[:])
            nc.sync.dma_start(x_sb[:], xsh[:])

            # ---- C one-hot cast bf16 -> f32r for the W-stage (DVE)
            for h in range(2):
                for i in range(2):
                    nc.vector.tensor_copy(
                        cf_t[h][:, i * gc // 2:(i + 1) * gc // 2],
                        c_t[h][:, i * gc // 2:(i + 1) * gc // 2])

            # ---- tanh(p'_i + q_j) per chunk (ACT), then mask (DVE strips)
            th_sb = pp.tile([128, AW], f32, tag="th", name="th")
            for c in range(NCH):
                nc.scalar.activation(th_sb[:, c * SH:(c + 1) * SH], pp_sb[:],
                                     mybir.ActivationFunctionType.Tanh,
                                     bias=q_sb[:, c:c + 1], scale=1.0)
            m_t = [pp.tile([128, ga], wdt, tag=f"mt{i}", name=f"mt{i}")
                   for i in range(NMASK)]
            for i in range(NMASK):
                nc.vector.tensor_tensor(m_t[i][:],
                                        th_sb[:, i * ga:(i + 1) * ga],
                                        a_t[i][:],
                                        op=mybir.AluOpType.mult)

            # ---- PT-stage (fp16): PT[j_local, k] = sum_i A[i, j] C[i, k]
            PT = [pp.tile([128, KPAD], wdt, tag=f"PT{m}", name=f"PT{m}")
                  for m in range(MCH)]
            for m in range(MCH):
                acc = ps.tile([128, KPAD], f32, tag="ptps", name="ptps")
                for c in range(NCH):
                    off = (c % 4) * SH + m * 128
                    nc.tensor.matmul(acc[:],
                                     a_t[c // 4][:, off:off + 128],
                                     c_t[c // 12][:, (c % 12) * KPAD:(c % 12 + 1) * KPAD],
                                     start=(c == 0), stop=(c == NCH - 1))
                nc.scalar.copy(PT[m][:], acc[:])

            # ---- W-stage (fp16): W[i_local, k] = sum_j M[j, i] C[j, k]
            Wt = [pp.tile([128, KPAD], wdt, tag=f"W{m}", name=f"W{m}")
                  for m in range(MCH)]
            for m in range(MCH):
                acc = ps.tile([128, KPAD], f32, tag="wps", name="wps")
                for c in range(NCH):
                    off = (c % 4) * SH + m * 128
                    nc.tensor.matmul(acc[:],
                                     m_t[c // 4][:, off:off + 128],
                                     cf_t[c // 12][:, (c % 12) * KPAD:(c % 12 + 1) * KPAD],
                                     start=(c == 0), stop=(c == NCH - 1))
                nc.scalar.copy(Wt[m][:], acc[:])

            # ---- X-stage (transposed, f32r): X.T[f, k] = sum_i x[i,f] W[i,k]
            xo_sb = pp.tile([128, KPAD], f32, tag="xout", name="xout")
            acc = ps.tile([128, KPAD], f32, tag="xps", name="xps")
            for m in range(MCH):
                nc.tensor.matmul(acc[:], x_sb[:, m, :], Wt[m][:],
                                 start=(m == 0), stop=(m == MCH - 1))
            nc.vector.tensor_copy(xo_sb[:], acc[:])
            nc.sync.dma_start(xpart[:], xo_sb[:])

            # ---- An-stage (fp16): An[k, l] = sum_j PT[j, k] Csh[j, l]
            an_sb = pp.tile([128, 2, KPAD], f32, tag="anout", name="anout")
            for kc in range(2):
                acc2 = ps.tile([128, KPAD], f32, tag="anps", name="anps")
                for m in range(MCH):
                    nc.tensor.matmul(acc2[:], PT[m][:, kc * 128:(kc + 1) * 128],
                                     csh_sb[:, m * KPAD:(m + 1) * KPAD],
                                     start=(m == 0), stop=(m == MCH - 1))
                nc.vector.tensor_copy(an_sb[:, kc, :], acc2[:])
                nc.sync.dma_start(anpart[:, kc, :], an_sb[:, kc, :])

    nc.compile()
    return nc


def _get_program():
    global _PROG
    if _PROG is None:
        _PROG = _build_program()
    return _PROG


def _core_inputs(r, x, p, q, b, A, cluster, single):
    import ml_dtypes
    bf = ml_dtypes.bfloat16
    cols = slice(r * SH, (r + 1) * SH)
    pp_ = (p[cols] + np.float32(b)).astype(np.float32)
    C = np.zeros((N, KPAD), dtype=bf)
    C[np.arange(N), cluster] = 1
    return {
        "abf": np.ascontiguousarray(
            A[:, cols].reshape(NCH, 128, SH).transpose(1, 0, 2)
        ).reshape(128, NCH * SH).astype(bf),
        "cbf": np.ascontiguousarray(
            C.reshape(NCH, 128, KPAD).transpose(1, 0, 2)).reshape(128, NCH * KPAD),
        "cshf": np.ascontiguousarray(
            C[cols].reshape(MCH, 128, KPAD).transpose(1, 0, 2)
        ).reshape(128, MCH * KPAD).astype(np.float32),
        "xsh": np.ascontiguousarray(
            x[cols].reshape(MCH, 128, F).transpose(1, 0, 2).astype(np.float32)),
        "ppmat": np.ascontiguousarray(
            np.broadcast_to(pp_[None, :], (128, SH)).astype(np.float32)),
        "qmat": np.ascontiguousarray(q.reshape(NCH, 128).T.astype(np.float32)),
    }


def kernel(x, edge_index, batch, w, b):
    x = np.asarray(x)
    batch = np.asarray(batch)
    p, q, A, cluster, K, single = _host_prep(x, edge_index, batch, w, b)
    if K > KPAD:
        return _numpy_fallback(x, batch, p, q, b, A, cluster, single)

    from concourse.bass_utils import run_bass_kernel_spmd
    nc = _get_program()
    in_maps = [_core_inputs(r, x, p, q, b, A, cluster, single)
               for r in range(NCORES)]
    res = run_bass_kernel_spmd(nc, in_maps, list(range(NCORES))).results

    Xp = np.zeros((KPAD, F), dtype=np.float32)
    Anp = np.zeros((KPAD, KPAD), dtype=np.float32)
    for r in range(NCORES):
        Xp += res[r]["xpart"].T
        Anp += res[r]["anpart"].transpose(1, 0, 2).reshape(KPAD, KPAD)
    np.fill_diagonal(Anp, 0.0)

    sidx = np.nonzero(single)[0]
    np.add.at(Xp, cluster[sidx], x[sidx].astype(np.float32))
    X_new = np.zeros((N, F), dtype=np.float32)
    X_new[:KPAD] = Xp
    A_new = np.zeros((N, N), dtype=np.float32)
    A_new[:KPAD, :KPAD] = Anp
    new_batch = np.zeros(N, dtype=batch.dtype)
    np.maximum.at(new_batch, cluster, batch)
    return X_new, A_new, new_batch, cluster.astype(np.int32)


# revision 16
# speedup vs baseline: 4.4854x; 1.1830x over previous
"""ClusterPooling kernel for 8x Trainium2 NeuronCores (Bass/Tile).

Decomposition (validated against the jax reference):
  e(u,v) = tanh(p_u + q_v + b),  p = x @ w[:F], q = x @ w[F:]
  A   = symmetric 0/1 adjacency (diag 0)        [dense, needed on device]
  S   = A * tanh(p+q+b) outer-sum mask          [dense, device]
  A_c = directed contracted adjacency           [sparse, host: sign tests only]
  labels = reference's directed min-label fixed point  [sparse O(E), host]
  cluster/C = consecutive relabel one-hot       [device builds C from ids]
  X_new = (S@C).T @ x ; A_new = C.T @ A @ C     [dense matmuls, device]

Sharding: core r owns columns R_r = [r*384, (r+1)*384) of the dense
matrices (column shard of A == row shard of A^T; A symmetric).  Each core
computes [256,128] / [256,256] partials; host sums the 8 partials and
embeds into the padded full-size outputs.
"""
import numpy as np

N, F, E = 3072, 128, 24576
NCORES = 8
SH = N // NCORES          # 384 shard columns per core
NCH = N // 128            # 24 partition chunks of the full dim
MCH = SH // 128           # 3 partition chunks of the shard dim
KPAD = 256                # padded cluster count (2 x 128)

_PROG = None              # cached compiled program


# ---------------------------------------------------------------- host math
def _host_prep(x, edge_index, batch, w, b):
    x = np.asarray(x, dtype=np.float32)
    w = np.asarray(w, dtype=np.float32)
    ei = np.asarray(edge_index)
    b = np.float32(np.asarray(b))

    src = ei[0].astype(np.int64)
    dst = ei[1].astype(np.int64)
    s2 = np.concatenate([src, dst])
    d2 = np.concatenate([dst, src])

    # Sign-critical per-edge scores: replicate the reference's exact ops on
    # jax CPU so threshold decisions bit-match the oracle.
    try:
        import jax
        import jax.numpy as jnp
        cpu = jax.devices("cpu")[0]
        with jax.default_device(cpu):
            xj = jnp.asarray(x)
            wj = jnp.asarray(w)
            t = xj[s2] @ wj[:F] + xj[d2] @ wj[F:] + jnp.asarray(b)
            contract = np.asarray(t > 0.0)
    except Exception:
        p_ = x @ w[:F]
        q_ = x @ w[F:]
        contract = (p_[s2] + q_[d2] + float(b)) > 0.0

    p = x @ w[:F]
    q = x @ w[F:]

    # symmetric adjacency mask, diag zero
    A = np.zeros((N, N), dtype=np.uint8)
    A[s2, d2] = 1
    np.fill_diagonal(A, 0)

    keep = contract & (s2 != d2)
    cs, cd = s2[keep], d2[keep]

    # reference's directed min-label propagation + pointer jumping, exactly
    labels = np.arange(N, dtype=np.int64)
    while True:
        neigh = np.full(N, N, dtype=np.int64)
        np.minimum.at(neigh, cs, labels[cd])
        new = np.minimum(labels, neigh)
        new = np.minimum(new, new[new])
        if np.array_equal(new, labels):
            break
        labels = new

    roots = (labels == np.arange(N)).astype(np.int64)
    cluster = (np.cumsum(roots) - 1)[labels]
    K = int(cluster.max()) + 1

    deg = np.zeros(N, dtype=np.int64)
    np.add.at(deg, cs, 1)
    np.add.at(deg, cd, 1)
    single = (deg == 0).astype(np.float32)

    return p, q, A, cluster, K, single


def _numpy_fallback(x, batch, p, q, b, A, cluster, single):
    """Dense-on-host path, used only if K > KPAD (never for the fixed input)."""
    K = int(cluster.max()) + 1
    C = np.zeros((N, K), dtype=np.float32)
    C[np.arange(N), cluster] = 1.0
    Tm = np.tanh((p + float(b))[None, :] + q[:, None]).astype(np.float32)
    M = A.astype(np.float32).T * Tm            # M[j,i] = S[i,j]
    W = M.T @ C + single[:, None] * C
    X = W.T @ x.astype(np.float32)
    An = C.T @ A.astype(np.float32) @ C
    np.fill_diagonal(An, 0.0)
    X_new = np.zeros((N, F), dtype=np.float32)
    X_new[:K] = X
    A_new = np.zeros((N, N), dtype=np.float32)
    A_new[:K, :K] = An
    new_batch = np.zeros(N, dtype=np.asarray(batch).dtype)
    np.maximum.at(new_batch, cluster, np.asarray(batch))
    return X_new, A_new, new_batch, cluster.astype(np.int32)


# ------------------------------------------------------------- bass program
USE_F32R = True      # single-pass fp32 matmuls for W/An stages (4x faster)
NDMA = 4             # acol8 DMA / cast split
NMASK = 6            # mask multiply split


def _build_program():
    import concourse.bacc as bacc
    import concourse.tile as tile
    import concourse.mybir as mybir

    f32 = mybir.dt.float32
    f32r = mybir.dt.float32r
    bf16 = mybir.dt.bfloat16
    f16 = mybir.dt.float16
    wdt = f32r if USE_F32R else f32

    nc = bacc.Bacc("TRN2", target_bir_lowering=False, debug=False,
                   num_devices=NCORES)

    abf = nc.dram_tensor("abf", [128, NCH * SH], bf16, kind="ExternalInput").ap()
    cbf = nc.dram_tensor("cbf", [128, NCH * KPAD], bf16, kind="ExternalInput").ap()
    cshf = nc.dram_tensor("cshf", [128, MCH * KPAD], wdt, kind="ExternalInput").ap()
    xsh = nc.dram_tensor("xsh", [128, MCH, F], wdt, kind="ExternalInput").ap()
    ppmat = nc.dram_tensor("ppmat", [128, SH], f32, kind="ExternalInput").ap()
    qmat = nc.dram_tensor("qmat", [128, NCH], f32, kind="ExternalInput").ap()

    xpart = nc.dram_tensor("xpart", [128, KPAD], f32, kind="ExternalOutput").ap()
    anpart = nc.dram_tensor("anpart", [128, 2, KPAD], f32, kind="ExternalOutput").ap()

    AW = NCH * SH          # 9216  A columns per partition
    CW = NCH * KPAD        # 6144  one-hot C width

    with tile.TileContext(nc) as tc:
        with (
            tc.tile_pool(name="persist", bufs=1) as pp,
            tc.tile_pool(name="psum", bufs=2, space="PSUM") as ps,
        ):
            # ---- inputs (small/early first: tanh can start immediately)
            # strips are separate tiles so consumers dep only on their strip
            pp_sb = pp.tile([128, SH], f32, tag="ppm", name="ppm")
            q_sb = pp.tile([128, NCH], f32, tag="qm", name="qm")
            ga = AW // NMASK
            gc = CW // 2
            a_t = [pp.tile([128, ga], bf16, tag=f"at{i}", name=f"at{i}")
                   for i in range(NMASK)]
            c_t = [pp.tile([128, gc], bf16, tag=f"ct{i}", name=f"ct{i}")
                   for i in range(2)]
            cf_t = [pp.tile([128, gc], wdt, tag=f"cft{i}", name=f"cft{i}")
                    for i in range(2)]
            csh_sb = pp.tile([128, MCH * KPAD], wdt, tag="csh", name="csh")
            x_sb = pp.tile([128, MCH, F], wdt, tag="xsh", name="xsh")
            # DMA order tuned so PT/W consumption can chase arrivals
            nc.sync.dma_start(pp_sb[:], ppmat[:])
            nc.sync.dma_start(q_sb[:], qmat[:])
            nc.sync.dma_start(c_t[0][:], cbf[:, 0:gc])
            nc.sync.dma_start(a_t[0][:], abf[:, 0:ga])
            nc.sync.dma_start(a_t[1][:], abf[:, ga:2 * ga])
            nc.sync.dma_start(c_t[1][:], cbf[:, gc:])
            for i in range(2, NMASK):
                nc.sync.dma_start(a_t[i][:], abf[:, i * ga:(i + 1) * ga])
            nc.sync.dma_start(csh_sb[:], cshf[:])
            nc.sync.dma_start(x_sb[:], xsh[:])

            # ---- C one-hot cast bf16 -> f32r for the W-stage (DVE)
            for h in range(2):
                for i in range(2):
                    nc.vector.tensor_copy(
                        cf_t[h][:, i * gc // 2:(i + 1) * gc // 2],
                        c_t[h][:, i * gc // 2:(i + 1) * gc // 2])

            # ---- tanh(p'_i + q_j) per chunk (ACT), then mask (DVE strips)
            th_sb = pp.tile([128, AW], f32, tag="th", name="th")
            for c in range(NCH):
                nc.scalar.activation(th_sb[:, c * SH:(c + 1) * SH], pp_sb[:],
                                     mybir.ActivationFunctionType.Tanh,
                                     bias=q_sb[:, c:c + 1], scale=1.0)
            m_t = [pp.tile([128, ga], wdt, tag=f"mt{i}", name=f"mt{i}")
                   for i in range(NMASK)]
            for i in range(NMASK):
                nc.vector.tensor_tensor(m_t[i][:],
                                        th_sb[:, i * ga:(i + 1) * ga],
                                        a_t[i][:],
                                        op=mybir.AluOpType.mult)

            # ---- PT-stage (fp16): PT[j_local, k] = sum_i A[i, j] C[i, k]
            PT = [pp.tile([128, KPAD], wdt, tag=f"PT{m}", name=f"PT{m}")
                  for m in range(MCH)]
            for m in range(MCH):
                acc = ps.tile([128, KPAD], f32, tag="ptps", name="ptps")
                for c in range(NCH):
                    off = (c % 4) * SH + m * 128
                    nc.tensor.matmul(acc[:],
                                     a_t[c // 4][:, off:off + 128],
                                     c_t[c // 12][:, (c % 12) * KPAD:(c % 12 + 1) * KPAD],
                                     start=(c == 0), stop=(c == NCH - 1))
                nc.scalar.copy(PT[m][:], acc[:])

            # ---- An-stage (fp16): An[k, l] = sum_j PT[j, k] Csh[j, l]
            an_sb = pp.tile([128, 2, KPAD], f32, tag="anout", name="anout")
            for kc in range(2):
                acc2 = ps.tile([128, KPAD], f32, tag="anps", name="anps")
                for m in range(MCH):
                    nc.tensor.matmul(acc2[:], PT[m][:, kc * 128:(kc + 1) * 128],
                                     csh_sb[:, m * KPAD:(m + 1) * KPAD],
                                     start=(m == 0), stop=(m == MCH - 1))
                nc.vector.tensor_copy(an_sb[:, kc, :], acc2[:])
                nc.sync.dma_start(anpart[:, kc, :], an_sb[:, kc, :])

            # ---- W-stage (fp16): W[i_local, k] = sum_j M[j, i] C[j, k]
            Wt = [pp.tile([128, KPAD], wdt, tag=f"W{m}", name=f"W{m}")
                  for m in range(MCH)]
            for m in range(MCH):
                acc = ps.tile([128, KPAD], f32, tag="wps", name="wps")
                for c in range(NCH):
                    off = (c % 4) * SH + m * 128
                    nc.tensor.matmul(acc[:],
                                     m_t[c // 4][:, off:off + 128],
                                     cf_t[c // 12][:, (c % 12) * KPAD:(c % 12 + 1) * KPAD],
                                     start=(c == 0), stop=(c == NCH - 1))
                nc.scalar.copy(Wt[m][:], acc[:])

            # ---- X-stage (transposed, f32r): X.T[f, k] = sum_i x[i,f] W[i,k]
            xo_sb = pp.tile([128, KPAD], f32, tag="xout", name="xout")
            acc = ps.tile([128, KPAD], f32, tag="xps", name="xps")
            for m in range(MCH):
                nc.tensor.matmul(acc[:], x_sb[:, m, :], Wt[m][:],
                                 start=(m == 0), stop=(m == MCH - 1))
            nc.vector.tensor_copy(xo_sb[:], acc[:])
            nc.sync.dma_start(xpart[:], xo_sb[:])

    nc.compile()
    return nc


def _get_program():
    global _PROG
    if _PROG is None:
        _PROG = _build_program()
    return _PROG


def _core_inputs(r, x, p, q, b, A, cluster, single):
    import ml_dtypes
    bf = ml_dtypes.bfloat16
    cols = slice(r * SH, (r + 1) * SH)
    pp_ = (p[cols] + np.float32(b)).astype(np.float32)
    C = np.zeros((N, KPAD), dtype=bf)
    C[np.arange(N), cluster] = 1
    return {
        "abf": np.ascontiguousarray(
            A[:, cols].reshape(NCH, 128, SH).transpose(1, 0, 2)
        ).reshape(128, NCH * SH).astype(bf),
        "cbf": np.ascontiguousarray(
            C.reshape(NCH, 128, KPAD).transpose(1, 0, 2)).reshape(128, NCH * KPAD),
        "cshf": np.ascontiguousarray(
            C[cols].reshape(MCH, 128, KPAD).transpose(1, 0, 2)
        ).reshape(128, MCH * KPAD).astype(np.float32),
        "xsh": np.ascontiguousarray(
            x[cols].reshape(MCH, 128, F).transpose(1, 0, 2).astype(np.float32)),
        "ppmat": np.ascontiguousarray(
            np.broadcast_to(pp_[None, :], (128, SH)).astype(np.float32)),
        "qmat": np.ascontiguousarray(q.reshape(NCH, 128).T.astype(np.float32)),
    }


def kernel(x, edge_index, batch, w, b):
    x = np.asarray(x)
    batch = np.asarray(batch)
    p, q, A, cluster, K, single = _host_prep(x, edge_index, batch, w, b)
    if K > KPAD:
        return _numpy_fallback(x, batch, p, q, b, A, cluster, single)

    from concourse.bass_utils import run_bass_kernel_spmd
    nc = _get_program()
    in_maps = [_core_inputs(r, x, p, q, b, A, cluster, single)
               for r in range(NCORES)]
    res = run_bass_kernel_spmd(nc, in_maps, list(range(NCORES))).results

    Xp = np.zeros((KPAD, F), dtype=np.float32)
    Anp = np.zeros((KPAD, KPAD), dtype=np.float32)
    for r in range(NCORES):
        Xp += res[r]["xpart"].T
        Anp += res[r]["anpart"].transpose(1, 0, 2).reshape(KPAD, KPAD)
    np.fill_diagonal(Anp, 0.0)

    sidx = np.nonzero(single)[0]
    np.add.at(Xp, cluster[sidx], x[sidx].astype(np.float32))
    X_new = np.zeros((N, F), dtype=np.float32)
    X_new[:KPAD] = Xp
    A_new = np.zeros((N, N), dtype=np.float32)
    A_new[:KPAD, :KPAD] = Anp
    new_batch = np.zeros(N, dtype=batch.dtype)
    np.maximum.at(new_batch, cluster, batch)
    return X_new, A_new, new_batch, cluster.astype(np.int32)
